# revision 39
# baseline (speedup 1.0000x reference)
"""Trainium2 Bass kernel for Swin-style windowed cosine attention.

Problem: nn_Attention_8100308321041
  q,k,v: [512, 8, 256, 16] f32; table: [961, 8]; index: [65536] i64;
  mask: [64, 256, 256] f32; out: [512, 256, 128] f32.

Strategy (8 NeuronCores, pure data-parallel):
  - Shard window-instances b by (b % 64) % 8 == core  -> 64 instances/core,
    ordered (wl, img) so each per-window bias+mask chunk is fetched once and
    reused across 8 images while the next chunk prefetches.
  - Host prep: l2-normalize q/k, transpose to a bf16 4-head row-group layout
    (partition 32*g + d) so the 4 heads of a "half" run as one 4-concurrent
    QK wave via tile_position row groups; gather table[index] -> bias and
    combine bias+mask into C, emitted three ways: Cb (bf16, raw, pair p0),
    Cp = A*C + B (fp32, Schraudolph, heads 2,3,6,7) and Ce = exp(C) (bf16,
    heads 2,3,4,5); build v_aug with a ones column (fused softmax denom).
  - Device per (window, head-pair), balancing PE/ScalarE/VectorE which are
    all within ~5%% of saturation:
      S'[m,n] = kT.T @ qT  (bf16 matmul, 2 m-chunks, 4-head row-group waves)
      path A (p0): C preloaded into PSUM via identity-stationary matmul,
        P' = exp(S'+C) on ScalarE (bf16, 2 heads per ACTIVATE)
      path B (p3 + p1 on odd insts): P' = bitcast_bf16(int16(A*S' + C'))
        on VectorE (Schraudolph exp, fused into one scalar_tensor_tensor)
      path C (p2 + p1 on even insts): P' = exp(S') on ScalarE (no C-add
        needed!) then P' *= exp(C) on VectorE (bf16 2x-mode tensor_tensor)
      out[n, 0:16|denom] = P'.T @ v_aug   (4 matmuls, K=128 chunks)
      out = out[:, :16] * (1/denom)       (VectorE, one op per window)
  - Emission is software-pipelined: instance i's AV matmuls + normalize are
    emitted after instance i+1's preload+QK so the strict-FIFO PE queue
    never head-of-line blocks on exp-dependent work.
"""

import os
import sys

sys.path.insert(0, "/opt/trn_rl_repo")

import numpy as np
import ml_dtypes

import concourse.bass as bass
import concourse.bacc as bacc
import concourse.mybir as mybir
from concourse import tile
from concourse.bass_utils import run_bass_kernel_spmd

BF16 = ml_dtypes.bfloat16
FP8 = ml_dtypes.float8_e4m3

B_, H, N, D = 512, 8, 256, 16
NW = 64          # windows per image
M_CORES = 8
IMG = B_ // NW   # 8 images
WL = NW // M_CORES  # 8 distinct windows per core
NI = IMG * WL    # 64 instances per core
HD = H * D       # 128
EPS = 1e-12
CB_WL = 2 * 2 * N      # additive-C cols per local window, heads 0-1 (1024)
CP_WL = 4 * 2 * N      # pre-scaled C' cols per local window, heads 2,3,6,7 (2048)
CE_WL = 6 * 2 * N      # exp(C) cols per local window, heads 0-5 (3072)
A16 = 128.0 / float(np.log(2.0))     # Schraudolph scale for bf16-via-int16
B16 = 127.0 * 128.0 - 5.09           # Schraudolph bias (round-to-nearest c)

_NC_CACHE = {}


def build_bass(trace_sim=False):
    nc = bacc.Bacc("TRN2", target_bir_lowering=False, debug=False, num_devices=M_CORES)
    # 4 heads per half-tile: partition 32*g + d, free (q|k, n) — bf16 keeps
    # FWL enabled so LDWEIGHTS (53ns) hides under the 107ns rhs stream, and
    # tile_position row groups let 4 heads' QK matmuls overlap on silicon.
    qk8 = nc.declare_dram_parameter("qk8", [NI, 128, 2 * 2 * N], mybir.dt.bfloat16, isOutput=False)
    vA = nc.declare_dram_parameter("vA", [NI, 128, 2 * H * 17], mybir.dt.bfloat16, isOutput=False)
    Cb = nc.declare_dram_parameter("Cb", [128, WL * CB_WL], mybir.dt.bfloat16, isOutput=False)
    Cp = nc.declare_dram_parameter("Cp", [128, WL * CP_WL], mybir.dt.float32, isOutput=False)
    Ce = nc.declare_dram_parameter("Ce", [128, WL * CE_WL], mybir.dt.bfloat16, isOutput=False)
    Ib = nc.declare_dram_parameter("Ib", [128, 128], mybir.dt.bfloat16, isOutput=False)
    out = nc.declare_dram_parameter("out", [NI, N, HD], mybir.dt.float32, isOutput=True)

    FP32 = mybir.dt.float32
    BF = mybir.dt.bfloat16
    I16 = mybir.dt.int16
    Exp = mybir.ActivationFunctionType.Exp
    DR = mybir.MatmulPerfMode.DoubleRow

    with tile.TileContext(nc, trace_sim=trace_sim) as tc:
        with (
            tc.tile_pool(name="const", bufs=1) as constp,
            tc.tile_pool(name="qk", bufs=6) as qkp,
            tc.tile_pool(name="vp", bufs=7) as vp,
            tc.tile_pool(name="pp", bufs=13) as ppool,
            tc.tile_pool(name="op", bufs=3) as opool,
            tc.tile_pool(name="ps", bufs=3, space=bass.MemorySpace.PSUM) as psp,
            tc.tile_pool(name="av", bufs=2, space=bass.MemorySpace.PSUM) as avp,
        ):
            ctile = constp.tile([128, WL * CB_WL], BF)
            cptile = constp.tile([128, WL * CP_WL], FP32)
            cetile = constp.tile([128, WL * CE_WL], BF)
            itile = constp.tile([128, 128], BF)
            nc.gpsimd.dma_start(itile[:], Ib[:])

            def fetch_cb(wl):
                nc.gpsimd.dma_start(ctile[:, wl * CB_WL:(wl + 1) * CB_WL], Cb[:, wl * CB_WL:(wl + 1) * CB_WL])

            def fetch_cpe(wl):
                nc.gpsimd.dma_start(cptile[:, wl * CP_WL:(wl + 1) * CP_WL], Cp[:, wl * CP_WL:(wl + 1) * CP_WL])
                nc.gpsimd.dma_start(cetile[:, wl * CE_WL:(wl + 1) * CE_WL], Ce[:, wl * CE_WL:(wl + 1) * CE_WL])

            def fetch_c(wl):
                fetch_cb(wl)
                fetch_cpe(wl)

            # qk/v fetches are issued 3 instances ahead so a per-window C
            # lump on the same Pool DMA FIFO is absorbed by buffer depth
            # instead of stalling the next instance's QK.
            qk_tiles = {}
            v_tiles = {}

            def issue_io(j):
                if j >= NI or j in qk_tiles:
                    return
                qt = qkp.tile([128, 2 * 2 * N], BF)
                vt = vp.tile([128, 2 * H * 17], BF)
                nc.gpsimd.dma_start(qt[:], qk8[j])
                nc.gpsimd.dma_start(vt[:], vA[j])
                qk_tiles[j] = qt
                v_tiles[j] = vt

            def emit_tail(inst, pexp, vtile):
                # AV + normalize + out DMA for instance `inst`; emitted after
                # the NEXT instance's preload+QK so the PE FIFO never blocks
                # on exp-dependent AV matmuls (software pipelining).
                avps = avp.tile([128, 2 * H * 17], FP32)
                for hp in (0, 3, 2, 1):
                    pbf = pexp[hp]
                    for hh in range(2):
                        h = 2 * hp + hh
                        hoff = hh * 512
                        for nck in range(2):
                            for mc in range(2):
                                nc.tensor.matmul(
                                    avps[:, nck * (H * 17) + h * 17: nck * (H * 17) + h * 17 + 17],
                                    pbf[:, hoff + mc * 256 + nck * 128: hoff + mc * 256 + nck * 128 + 128],
                                    vtile[:, mc * (H * 17) + h * 17: mc * (H * 17) + h * 17 + 17],
                                    start=(mc == 0), stop=(mc == 1),
                                )

                otile = opool.tile([128, 2 * HD], FP32)
                rtile = opool.tile([128, 2 * H], FP32, tag="recip")
                av3 = avps[:].rearrange("p (nck h x) -> p nck h x", nck=2, h=H)
                nc.vector.reciprocal(
                    rtile[:].rearrange("p (nck h) -> p nck h", nck=2),
                    av3[:, :, :, 16],
                )
                nc.vector.tensor_mul(
                    otile[:].rearrange("p (nck h d) -> p nck h d", nck=2, h=H),
                    av3[:, :, :, 0:D],
                    rtile[:].rearrange("p (nck h) -> p nck h", nck=2)[:, :, :, None].broadcast_to([128, 2, H, D]),
                )
                # single out DMA: dram[(nck*128+p), hd] <- otile[p, (nck, hd)]
                nc.sync.dma_start(
                    out[inst].rearrange("(nck p) hd -> p nck hd", nck=2),
                    otile[:].rearrange("p (nck hd) -> p nck hd", nck=2),
                )

            # startup order: Cb(0) (gates inst 0's preload) before the bulk
            # Cp/Ce so the first instances' qk/v fetches aren't queued
            # behind 3.5 MiB of C data.
            fetch_cb(0)
            issue_io(0)
            fetch_cpe(0)
            issue_io(1)
            issue_io(2)
            fetch_cb(1)
            issue_io(3)
            issue_io(4)
            fetch_cpe(1)
            pending = None
            for inst in range(NI):
                wl = inst // IMG
                issue_io(inst + 4)
                if inst % IMG == 2 and wl + 2 < WL:
                    fetch_c(wl + 2)
                qktile = qk_tiles.pop(inst)
                vtile = v_tiles.pop(inst)
                # partitions 32g+d hold heads h=4*half+g: free [half=2, qk=2, n=256]
                qk5 = qktile[:].rearrange("p (s q n) -> p s q n", s=2, q=2)

                # pair hp covers heads (2hp, 2hp+1); half h//4 maps 4 heads
                # onto 4 distinct PE row groups so QK waves run 4-concurrent.
                # Per-pair exp path: A = PE C-preload + ScalarE exp;
                # B = DVE Schraudolph (fused A*S + C' -> int16 bitcast bf16);
                # C = ScalarE exp(S) then DVE bf16 mult by exp(C) (2x mode).
                # path mix tuned to balance Tensor/Vector/Scalar busy time:
                # p0 skips its PE preload (path C) on 1/8 of instances, p1 is
                # C on 11/16; last instance splits exps across both engines
                # to shorten the drain tail.
                if inst == NI - 1:
                    p1_path = "B"
                else:
                    p1_path = "B" if inst % 16 in (0, 2, 4, 6, 8) else "C"
                p0_path = "C" if inst % 8 == 3 else "A"
                paths = {0: p0_path, 1: p1_path, 2: "C", 3: "B"}
                pstate = {}
                pexp = {}
                for half in (0, 1):
                    # half1 allocates p3 before p2: the 4th PSUM tile (bufs=3)
                    # waits on exp(p0); pinning that wait to p2 (whose ScalarE
                    # exp is last anyway) keeps p3's early DVE path unblocked.
                    pair_order = (0, 1) if half == 0 else (3, 2)
                    g_order = (0, 1, 2, 3) if half == 0 else (2, 3, 0, 1)
                    for hp in pair_order:
                        ps = psp.tile([128, 1024], FP32)
                        pstate[hp] = ps
                        if paths[hp] == "A":
                            for hh in range(2):
                                coff = wl * CB_WL + (2 * hp + hh) * 2 * N
                                nc.tensor.matmul(
                                    ps[:, hh * 512: hh * 512 + 512],
                                    itile[:],
                                    ctile[:, coff: coff + 512],
                                    start=True, stop=False,
                                    skip_group_check=True,
                                )
                    # QK: mc outer, row-group inner -> 2 waves of 4 concurrent
                    for mc in range(2):
                        for g in g_order:
                            h = 4 * half + g
                            hp = h // 2
                            ps = pstate[hp]
                            hoff = (h % 2) * 512
                            qkh = qk5[32 * g: 32 * g + D, half]
                            nc.tensor.matmul(
                                ps[:, hoff + mc * 256: hoff + mc * 256 + 256],
                                qkh[:, 1, mc * 128:(mc + 1) * 128],
                                qkh[:, 0, :],
                                start=(paths[hp] != "A"), stop=(mc == 1),
                                skip_group_check=True,
                                tile_position=(32 * g, 0),
                            )
                    # exp emission: p1's ScalarE path is deferred until after
                    # p2's, so ACT(p2) -- which frees the PSUM tile the next
                    # instance's QK waits on -- runs earlier on the ScalarE
                    # FIFO. DVE ops (stt) stay early.
                    if half == 0:
                        hp_list = (0,) if paths[1] == "C" else (0, 1)
                    else:
                        hp_list = (3, 2, 1) if paths[1] == "C" else (3, 2)
                    for hp in hp_list:
                        ps = pstate[hp]
                        if paths[hp] == "B":
                            ptile = ppool.tile([128, 1024], I16, tag="pt")
                            poff = wl * CP_WL + (0 if hp == 1 else 1) * 1024
                            nc.vector.scalar_tensor_tensor(
                                ptile[:], ps[:], A16, cptile[:, poff: poff + 1024],
                                mybir.AluOpType.mult, mybir.AluOpType.add,
                            )
                            pexp[hp] = ptile[:].bitcast(BF)
                        else:
                            ptile = ppool.tile([128, 1024], BF, tag="pt")
                            nc.scalar.activation(ptile[:], ps[:], Exp)
                            if paths[hp] == "C":
                                ceoff = wl * CE_WL + hp * 1024
                                p2 = ppool.tile([128, 1024], BF, tag="pt")
                                nc.vector.tensor_mul(
                                    p2[:], ptile[:], cetile[:, ceoff: ceoff + 1024],
                                )
                                ptile = p2
                            pexp[hp] = ptile[:]
                if pending is not None:
                    emit_tail(*pending)
                pending = (inst, pexp, vtile)
            emit_tail(*pending)
    nc.compile()
    return nc


def _host_prep(q, k, v, table, index, mask):
    """Returns per-core input maps + the inverse b-index map."""
    # l2 normalize q, k (host): matches F.normalize(x, dim=-1)
    qn = q / np.maximum(np.sqrt((q * q).sum(-1, keepdims=True)), EPS)
    kn = k / np.maximum(np.sqrt((k * k).sum(-1, keepdims=True)), EPS)
    # 4-head row-group layout: [b, g, d(padded to 32), half, qk, n], h = 4*half+g
    qk8 = np.zeros((B_, 4, 32, 2, 2, N), np.float32)
    qk8[:, :, :D, :, 0] = qn.transpose(0, 1, 3, 2).reshape(B_, 2, 4, D, N).transpose(0, 2, 3, 1, 4)
    qk8[:, :, :D, :, 1] = kn.transpose(0, 1, 3, 2).reshape(B_, 2, 4, D, N).transpose(0, 2, 3, 1, 4)
    qk8 = qk8.reshape(B_, 128, 2 * 2 * N).astype(BF16)
    # v_aug [b, n, h, 17] -> [b, mc, 128, h, 17] -> [b, 128, mc*h*17]
    vA = np.empty((B_, N, H, 17), np.float32)
    vA[..., :16] = v.transpose(0, 2, 1, 3)
    vA[..., 16] = 1.0
    vA = vA.reshape(B_, 2, 128, H * 17).transpose(0, 2, 1, 3).reshape(B_, 128, 2 * H * 17).astype(BF16)
    # bias'[h, m, n] = table[index[n*256+m], h]
    bias = table[index.astype(np.int64)].reshape(N, N, H).transpose(2, 1, 0)  # [h, m, n]
    maskT = mask.transpose(0, 2, 1)  # [w, m, n]

    in_maps = []
    b_order = []
    ident = np.eye(128, dtype=BF16)
    for c in range(M_CORES):
        # device instance i <-> wl = i // IMG, img = i % IMG
        bs = np.array([img * NW + (c + M_CORES * wl) for wl in range(WL) for img in range(IMG)])
        b_order.append(bs)
        # C[wl, h, m, n] = bias'[h] + maskT[c + 8*wl]
        C = (bias[None, :, :, :] + maskT[c::M_CORES][:, None, :, :]).astype(np.float32)
        C = C.reshape(WL, H, 2, 128, N)
        # additive path (pair p0): heads 0,1 bf16, [p, (wl h mc n)]
        Cb_ = C[:, :2].transpose(3, 0, 1, 2, 4).reshape(128, WL * CB_WL).astype(BF16)
        # Schraudolph path (pairs p1/p3): heads 2,3,6,7 fp32 pre-scaled A*C + B
        Cp_ = (A16 * C[:, [2, 3, 6, 7]] + B16).transpose(3, 0, 1, 2, 4).reshape(128, WL * CP_WL).astype(np.float32)
        # expC path (pairs p0/p1/p2): heads 0-5 bf16 exp(C)
        Ce_ = np.exp(C[:, :6]).transpose(3, 0, 1, 2, 4).reshape(128, WL * CE_WL).astype(BF16)
        in_maps.append({
            "qk8": np.ascontiguousarray(qk8[bs]),
            "vA": np.ascontiguousarray(vA[bs]),
            "Cb": Cb_,
            "Cp": Cp_,
            "Ce": Ce_,
            "Ib": ident,
        })
    return in_maps, b_order


def kernel(q, k, v, table, index, mask):
    q = np.asarray(q, np.float32)
    k = np.asarray(k, np.float32)
    v = np.asarray(v, np.float32)
    table = np.asarray(table, np.float32)
    index = np.asarray(index)
    mask = np.asarray(mask, np.float32)

    in_maps, b_order = _host_prep(q, k, v, table, index, mask)

    if "nc" not in _NC_CACHE:
        _NC_CACHE["nc"] = build_bass()
    nc = _NC_CACHE["nc"]

    res = run_bass_kernel_spmd(nc, in_maps, core_ids=list(range(M_CORES)))
    out = np.empty((B_, N, HD), np.float32)
    for c in range(M_CORES):
        out[b_order[c]] = res.results[c]["out"]
    return out


if __name__ == "__main__":
    rng = np.random.default_rng(0)
    q = rng.standard_normal((B_, H, N, D), dtype=np.float32)
    k = rng.standard_normal((B_, H, N, D), dtype=np.float32)
    v = rng.standard_normal((B_, H, N, D), dtype=np.float32)
    table = rng.standard_normal((961, H), dtype=np.float32)
    index = rng.integers(0, 961, size=(N * N,)).astype(np.int64)
    mask = rng.standard_normal((NW, N, N), dtype=np.float32)
    o = kernel(q=q, k=k, v=v, table=table, index=index, mask=mask)
    print("out", o.shape, o.dtype, float(np.abs(o).mean()))

